# revision 20
# baseline (speedup 1.0000x reference)
"""Trainium2 Bass kernel for BinaryMemoryRNN (scatter_memory) — fp8 edition.

Math (per batch row b):
    logits = h_prev @ M_w.T + M_b                 [B, 10]
    bits   = (sigmoid(logits) > 0.5) = (logits > -M_b)
    index  = sum(bits * 2^(9-i))                  [B] in [0, 1023]
    h_mem  = memory[index]
    pre    = x @ W_w.T + W_b + h_prev @ U_w.T + U_b + h_mem @ Q_w.T + Q_b
    out    = sigmoid(LayerNorm(pre) * ln_g + ln_b)

Key transforms vs the bf16 baseline (202us):
  * All heavy matmuls run in fp8e4 with MatmulPerfMode.DoubleRow (2 k-subtiles
    per instruction, 2x PE throughput vs bf16).
  * The address logits need ~1e-5 absolute accuracy (a flipped sign bit picks a
    different memory row).  h_prev is decomposed into a 3-level fp8 ladder
    (scales 1, 16, 256) and M_w into a 4-level ladder; logits are rebuilt from
    the 6 cross terms with i+j<=2.  Level 1 of the h ladder doubles as the
    activation operand of the U matmul.  Numpy-sim of this exact scheme on the
    harness inputs: rel_err 1.52e-2 (gate 2e-2).
  * R = memory @ Q_w.T (+bias) is precomputed per core (fp8 DoubleRow), stored
    bf16 in DRAM, and row-gathered by index — replaces gather+matmul.
  * pre is staged fp16, output written fp16; LayerNorm rstd via one batched
    magic-rsqrt per 4-tile group (vector-op count matters, not flops).

Sharding: data-parallel over batch across 8 cores (2048 rows each); weights +
memory table replicated.  Host pre-tiles activations as [kp, g, kc, 512]
(4 groups of 4 128-row tiles) so DoubleRow can slice k-chunk pairs directly.
"""

import numpy as np
import ml_dtypes
from contextlib import ExitStack

import concourse.bass as bass
import concourse.mybir as mybir
import concourse.tile as tile
from concourse import bacc
from concourse import bass_utils

P = 128            # partitions
NCORES = 8
B = 16384          # full batch
BC = B // NCORES   # batch rows per core (2048)
BT = BC // P       # b-tiles per core (16)
GT = 4             # tiles per group
NG = BT // GT      # groups per core (4)
KC = 8             # contraction chunks (1024 / 128)
H = 1024
NB = 10            # address bits
MEM = 1024         # memory rows
LN_EPS = 1e-5

F32 = mybir.dt.float32
F16 = mybir.dt.float16
BF16 = mybir.dt.bfloat16
F8 = mybir.dt.float8e4
I32 = mybir.dt.int32
BF16_NP = ml_dtypes.bfloat16
F8_NP = ml_dtypes.float8_e4m3
DR = mybir.MatmulPerfMode.DoubleRow

_CACHE = {}

# (i, j) cross terms of the h/Mw fp8 ladders and their descale factors
LPAIRS = [(i, j, 16.0 ** -(i + j)) for i in range(3) for j in range(4)
          if i + j <= 2]


def _bcast_ap(handle, n):
    """[n] DRAM tensor -> [P, n] AP broadcast across partitions (step 0)."""
    h = handle.ap()
    return bass.AP(tensor=h.tensor, offset=h.offset, ap=[[0, P], *list(h.ap)])


def build_nc(zero_affine=True, zero_cb=True, warmup=False):
    nc = bacc.Bacc("TRN2", debug=False, enable_asserts=False)

    x1 = nc.dram_tensor("x1", [P, NG, KC, GT * P], F8, kind="ExternalInput")
    h1 = nc.dram_tensor("h1", [P, NG, KC, GT * P], F8, kind="ExternalInput")
    h2 = nc.dram_tensor("h2", [P, NG, KC, GT * P], F8, kind="ExternalInput")
    h3 = nc.dram_tensor("h3", [P, NG, KC, GT * P], F8, kind="ExternalInput")
    W8 = nc.dram_tensor("W8", [P, KC, H], F8, kind="ExternalInput")
    U8 = nc.dram_tensor("U8", [P, KC, H], F8, kind="ExternalInput")
    Q8 = nc.dram_tensor("Q8", [P, KC, H], F8, kind="ExternalInput")
    Mem8 = nc.dram_tensor("Mem8", [P, KC, MEM], F8, kind="ExternalInput")
    # 4 Mw ladder levels padded to 32-partition blocks (DVE reads of the
    # matmul output must start at partition offsets that are multiples of 32)
    Mwp = nc.dram_tensor("Mwp", [P, KC, 4 * 32], F8, kind="ExternalInput")
    cb = nc.dram_tensor("cb", [H], F32, kind="ExternalInput")
    lng = nc.dram_tensor("lng", [H], F32, kind="ExternalInput")
    lnb = nc.dram_tensor("lnb", [H], F32, kind="ExternalInput")
    negmb = nc.dram_tensor("negmb", [NB], F32, kind="ExternalInput")
    powers = nc.dram_tensor("powers", [NB], F32, kind="ExternalInput")
    y = nc.dram_tensor("y", [BC, H], F16, kind="ExternalOutput")
    R = nc.dram_tensor("Rtab", [MEM, H], BF16, kind="Internal")
    wsink_d = nc.dram_tensor("wsink", [P, 1], F32, kind="Internal")
    y_ap = y.ap()
    R_ap = R.ap()

    with tile.TileContext(nc) as tc, ExitStack() as ctx:
        wpool = ctx.enter_context(tc.tile_pool(name="weights", bufs=1))
        work = ctx.enter_context(tc.tile_pool(name="work", bufs=4))
        gpool = ctx.enter_context(tc.tile_pool(name="gpool", bufs=3))
        epil = ctx.enter_context(tc.tile_pool(name="epil", bufs=GT + 2))
        small = ctx.enter_context(tc.tile_pool(name="small", bufs=2))
        psum = ctx.enter_context(tc.tile_pool(name="psum", bufs=2, space="PSUM"))

        if warmup:
            wu_l = wpool.tile([P, P], BF16)
            wu_r = wpool.tile([P, 512], BF16)
            nc.vector.memset(wu_l[:], 0)
            nc.vector.memset(wu_r[:], 0)
            ps_w = psum.tile([P, 512], F32, tag="psT", space="PSUM")
            for _ in range(12):
                nc.tensor.matmul(out=ps_w[:], lhsT=wu_l[:], rhs=wu_r[:],
                                 start=True, stop=True)
            wsink = wpool.tile([P, 1], F32)
            nc.vector.tensor_copy(out=wsink[:], in_=ps_w[:, 0:1])
            nc.sync.dma_start(out=wsink_d.ap()[:, :], in_=wsink[:])

        # ---- resident constants; mem on sync queue, q on vector queue so the
        # first DoubleRow pair unlocks after ~2 chunk transfers ----
        mem_sb = wpool.tile([P, KC, MEM], F8)
        q_sb = wpool.tile([P, KC, H], F8)
        w_sb = wpool.tile([P, KC, H], F8)
        u_sb = wpool.tile([P, KC, H], F8)
        mwp_sb = wpool.tile([P, KC, 4 * 32], F8)
        nc.sync.dma_start(out=mem_sb[:, 0:2, :], in_=Mem8.ap()[:, 0:2, :])
        nc.gpsimd.dma_start(out=q_sb[:, 0:2, :], in_=Q8.ap()[:, 0:2, :])
        nc.sync.dma_start(out=mem_sb[:, 2:KC, :], in_=Mem8.ap()[:, 2:KC, :])
        nc.gpsimd.dma_start(out=q_sb[:, 2:KC, :], in_=Q8.ap()[:, 2:KC, :])
        nc.gpsimd.dma_start(out=mwp_sb[:], in_=Mwp.ap()[:, :, :])
        nc.sync.dma_start(out=w_sb[:], in_=W8.ap()[:, :, :])
        nc.gpsimd.dma_start(out=u_sb[:], in_=U8.ap()[:, :, :])

        nmb_c = wpool.tile([NB, 1], F32)
        nc.gpsimd.dma_start(out=nmb_c[:], in_=negmb.ap()[:, None])
        pw_c = wpool.tile([NB, 1], F32)
        nc.gpsimd.dma_start(out=pw_c[:], in_=powers.ap()[:, None])
        ones10 = wpool.tile([NB, 1], F16)
        nc.vector.memset(ones10[:], 1.0)
        ident1 = wpool.tile([1, 1], F32)
        nc.vector.memset(ident1[:], 1.0)
        if not zero_cb:
            cbb = wpool.tile([P, H], F32)
            nc.gpsimd.dma_start(out=cbb[:], in_=_bcast_ap(cb, H))
        if not zero_affine:
            gb = wpool.tile([P, H], F32)
            bb = wpool.tile([P, H], F32)
            nc.gpsimd.dma_start(out=gb[:], in_=_bcast_ap(lng, H))
            nc.gpsimd.dma_start(out=bb[:], in_=_bcast_ap(lnb, H))

        # ---- phase 1: R = memory @ Q_w.T (+bias) -> DRAM bf16 ----
        # kc-pair outer over all 8 PSUM banks so each arriving chunk pair
        # unlocks 8 DoubleRow matmuls.
        tags8 = ["psA", "psB", "psL", "psT"] * 2
        for half in range(2):
            hs = slice(half * 512, (half + 1) * 512)
            ps_r = [psum.tile([P, 512], F32, tag=tags8[mt], space="PSUM",
                              name=f"psr{half}_{mt}") for mt in range(KC)]
            for kcp in range(KC // 2):
                ks = slice(2 * kcp, 2 * kcp + 2)
                for mt in range(KC):
                    nc.tensor.matmul(out=ps_r[mt][:],
                                     lhsT=mem_sb[:, ks, mt * P:(mt + 1) * P],
                                     rhs=q_sb[:, ks, hs],
                                     start=(kcp == 0), stop=(kcp == KC // 2 - 1),
                                     perf_mode=DR)
            for mt in range(KC):
                r_sb = work.tile([P, 512], BF16, tag="rtile")
                if zero_cb:
                    nc.scalar.copy(out=r_sb[:], in_=ps_r[mt][:])
                else:
                    nc.vector.tensor_add(out=r_sb[:], in0=ps_r[mt][:],
                                         in1=cbb[:, hs])
                nc.gpsimd.dma_start(out=R_ap[mt * P:(mt + 1) * P, hs],
                                    in_=r_sb[:])

        # ---- phase 2: groups of 4 tiles, logits pipelined one group ahead --
        def load_group(g):
            xg = gpool.tile([P, KC, GT * P], F8, tag="xg", name=f"xg{g}")
            hg = [gpool.tile([P, KC, GT * P], F8, tag=f"hg{i}",
                             name=f"hg{g}_{i}")
                  for i in range(3)]
            nc.scalar.dma_start(out=xg[:], in_=x1.ap()[:, g, :, :])
            for i, hsrc in enumerate((h1, h2, h3)):
                nc.scalar.dma_start(out=hg[i][:], in_=hsrc.ap()[:, g, :, :])
            return xg, hg

        def logits_group(g, hg):
            """3 fp8-ladder DR rounds + 6-pair combine -> per-tile indices."""
            lacc = small.tile([NB, 512], F32, tag="lacc", name=f"lacc{g}")
            for i in range(3):
                psLT = psum.tile([P, 512], F32, tag="psL", space="PSUM",
                                 name=f"psL{g}_{i}")
                for kcp in range(KC // 2):
                    ks = slice(2 * kcp, 2 * kcp + 2)
                    nc.tensor.matmul(out=psLT[:], lhsT=mwp_sb[:, ks, :],
                                     rhs=hg[i][:, ks, :],
                                     start=(kcp == 0),
                                     stop=(kcp == KC // 2 - 1),
                                     perf_mode=DR)
                for ii, jj, sc in LPAIRS:
                    if ii != i:
                        continue
                    blk = psLT[jj * 32:jj * 32 + NB, :]
                    if ii == 0 and jj == 0:
                        nc.scalar.copy(out=lacc[:], in_=blk)
                    else:
                        nc.vector.scalar_tensor_tensor(
                            out=lacc[:], in0=blk, scalar=sc, in1=lacc[:],
                            op0=mybir.AluOpType.mult, op1=mybir.AluOpType.add)

            bits = small.tile([NB, 512], F16, tag="bits", name=f"bits{g}")
            nc.vector.tensor_scalar(out=bits[:], in0=lacc[:],
                                    scalar1=nmb_c[:], scalar2=pw_c[:],
                                    op0=mybir.AluOpType.is_gt,
                                    op1=mybir.AluOpType.mult)
            psI = psum.tile([1, 512], F32, tag="psL", space="PSUM",
                            name=f"psI{g}")
            nc.tensor.matmul(out=psI[:], lhsT=ones10[:], rhs=bits[:],
                             start=True, stop=True)
            idxf = small.tile([1, 512], F32, tag="idxf", name=f"idxf{g}")
            nc.scalar.copy(out=idxf[:], in_=psI[:])
            psT = psum.tile([P, GT], F32, tag="psT", space="PSUM",
                            name=f"psT{g}")
            for t in range(GT):
                nc.tensor.transpose(out=psT[:, t:t + 1],
                                    in_=idxf[0:1, t * P:(t + 1) * P],
                                    identity=ident1[:])
            idx4 = small.tile([P, GT], I32, tag="idx4", name=f"idx4{g}")
            nc.vector.tensor_copy(out=idx4[:], in_=psT[:])
            return idx4

        def wu_tile(g, t, xg, hg0, rg, mvg):
            ps0 = psum.tile([P, 512], F32, tag="psA", space="PSUM",
                            name=f"ps0_{g}_{t}")
            ps1 = psum.tile([P, 512], F32, tag="psB", space="PSUM",
                            name=f"ps1_{g}_{t}")
            ts_ = slice(t * P, (t + 1) * P)
            for kcp in range(KC // 2):
                ks = slice(2 * kcp, 2 * kcp + 2)
                nc.tensor.matmul(out=ps0[:], lhsT=xg[:, ks, ts_],
                                 rhs=w_sb[:, ks, 0:512],
                                 start=(kcp == 0), stop=False, perf_mode=DR)
                nc.tensor.matmul(out=ps1[:], lhsT=xg[:, ks, ts_],
                                 rhs=w_sb[:, ks, 512:1024],
                                 start=(kcp == 0), stop=False, perf_mode=DR)
            for kcp in range(KC // 2):
                ks = slice(2 * kcp, 2 * kcp + 2)
                last = kcp == KC // 2 - 1
                nc.tensor.matmul(out=ps0[:], lhsT=hg0[:, ks, ts_],
                                 rhs=u_sb[:, ks, 0:512],
                                 start=False, stop=last, perf_mode=DR)
                nc.tensor.matmul(out=ps1[:], lhsT=hg0[:, ks, ts_],
                                 rhs=u_sb[:, ks, 512:1024],
                                 start=False, stop=last, perf_mode=DR)

            pre = epil.tile([P, H], F16, tag="pre", name=f"pre{g}_{t}")
            nc.vector.tensor_add(out=pre[:, 0:512], in0=ps0[:],
                                 in1=rg[:, 0:512])
            nc.vector.tensor_add(out=pre[:, 512:1024], in0=ps1[:],
                                 in1=rg[:, 512:1024])
            stats = small.tile([P, 2, 6], F32, tag="stats",
                               name=f"stats{g}_{t}")
            nc.vector.bn_stats(out=stats[:, 0, :], in_=pre[:, 0:512])
            nc.vector.bn_stats(out=stats[:, 1, :], in_=pre[:, 512:1024])
            nc.vector.bn_aggr(out=mvg[:, t, :], in_=stats[:])
            return pre

        def rstd_pair(g, p, mvg):
            """magic rsqrt + 1 Newton iteration, batched over a tile pair."""
            mvp = mvg[:, 2 * p:2 * p + 2, :]
            v2 = small.tile([P, 2], F32, tag="v2", name=f"v2_{g}_{p}")
            ri2 = small.tile([P, 2], I32, tag="ri2", name=f"ri2_{g}_{p}")
            t2 = small.tile([P, 2], F32, tag="t2", name=f"t2_{g}_{p}")
            nmr2 = small.tile([P, 2], F32, tag="nmr2", name=f"nmr2_{g}_{p}")
            ry = ri2[:].bitcast(F32)
            nc.vector.tensor_scalar_add(out=v2[:], in0=mvp[:, :, 1],
                                        scalar1=LN_EPS)
            nc.vector.tensor_scalar(out=ri2[:], in0=v2[:].bitcast(I32),
                                    scalar1=1, scalar2=None,
                                    op0=mybir.AluOpType.arith_shift_right)
            nc.vector.tensor_scalar(out=ri2[:], in0=ri2[:], scalar1=0,
                                    scalar2=None,
                                    op0=mybir.AluOpType.bitwise_not)
            nc.vector.tensor_scalar(out=ri2[:], in0=ri2[:],
                                    scalar1=0x5F3759E0, scalar2=None,
                                    op0=mybir.AluOpType.add)
            nc.vector.tensor_tensor(out=t2[:], in0=ry, in1=ry,
                                    op=mybir.AluOpType.mult)
            nc.vector.tensor_tensor(out=t2[:], in0=t2[:], in1=v2[:],
                                    op=mybir.AluOpType.mult)
            nc.vector.tensor_scalar(out=t2[:], in0=t2[:], scalar1=-0.5,
                                    scalar2=1.5, op0=mybir.AluOpType.mult,
                                    op1=mybir.AluOpType.add)
            nc.vector.tensor_tensor(out=ry, in0=ry, in1=t2[:],
                                    op=mybir.AluOpType.mult)
            nc.vector.scalar_tensor_tensor(out=nmr2[:], in0=mvp[:, :, 0],
                                           scalar=-1.0, in1=ry,
                                           op0=mybir.AluOpType.mult,
                                           op1=mybir.AluOpType.mult)
            return ri2, nmr2

        def finish_tile(g, t, pre, ri2, nmr2):
            tp = t % 2
            rys = ri2[:, tp:tp + 1].bitcast(F32)
            if zero_affine:
                ob = work.tile([P, H], F16, tag="ob", name=f"ob{g}_{t}")
                nc.scalar.activation(
                    out=ob[:], in_=pre[:],
                    func=mybir.ActivationFunctionType.Sigmoid,
                    bias=nmr2[:, tp:tp + 1], scale=rys)
            else:
                # norm*g + b = (pre*rstd)*g + (nmr*g + b)
                nrm = epil.tile([P, H], F32, tag="nrm", name=f"nrm{g}_{t}")
                off = epil.tile([P, H], F32, tag="off", name=f"off{g}_{t}")
                nc.vector.scalar_tensor_tensor(
                    out=nrm[:], in0=pre[:], scalar=rys,
                    op0=mybir.AluOpType.mult, in1=gb[:],
                    op1=mybir.AluOpType.mult)
                nc.vector.scalar_tensor_tensor(
                    out=off[:], in0=gb[:], scalar=nmr2[:, tp:tp + 1],
                    op0=mybir.AluOpType.mult, in1=bb[:],
                    op1=mybir.AluOpType.add)
                nc.vector.tensor_add(out=nrm[:], in0=nrm[:], in1=off[:])
                ob = work.tile([P, H], F16, tag="ob", name=f"ob{g}_{t}")
                nc.scalar.activation(
                    out=ob[:], in_=nrm[:],
                    func=mybir.ActivationFunctionType.Sigmoid)
            bt = g * GT + t
            nc.sync.dma_start(out=y_ap[bt * P:(bt + 1) * P, :], in_=ob[:])

        grp = {0: load_group(0), 1: load_group(1)}
        idx4s = {0: logits_group(0, grp[0][1])}

        for g in range(NG):
            xg, hg = grp[g]
            idx4 = idx4s[g]
            rgs = []
            for t in range(GT):
                rg = work.tile([P, H], BF16, tag="rg", name=f"rg{g}_{t}")
                nc.gpsimd.indirect_dma_start(
                    out=rg[:], out_offset=None, in_=R_ap[:, :],
                    in_offset=bass.IndirectOffsetOnAxis(ap=idx4[:, t:t + 1],
                                                        axis=0))
                rgs.append(rg)

            # next group's logits go FIRST: the vector queue is in-order, so
            # they must be queued ahead of this group's (gather-blocked)
            # epilogue ops to overlap the index->gather chain.
            if g + 1 < NG:
                idx4s[g + 1] = logits_group(g + 1, grp[g + 1][1])
            if g + 2 < NG:
                grp[g + 2] = load_group(g + 2)

            mvg = small.tile([P, GT, 2], F32, tag="mvg", name=f"mvg{g}")
            for p in range(2):
                pres = [wu_tile(g, t, xg, hg[0], rgs[t], mvg)
                        for t in (2 * p, 2 * p + 1)]
                ri2, nmr2 = rstd_pair(g, p, mvg)
                for t in (2 * p, 2 * p + 1):
                    finish_tile(g, t, pres[t % 2], ri2, nmr2)

    nc.compile()
    return nc


import os as _os

FLAGS = {
    "warmup": bool(int(_os.environ.get("K_WARMUP", "1"))),
}


def _get_nc(zero_affine=True, zero_cb=True):
    key = ("nc", zero_affine, zero_cb, tuple(sorted(FLAGS.items())))
    if key not in _CACHE:
        _CACHE[key] = build_nc(zero_affine, zero_cb=zero_cb, **FLAGS)
    return _CACHE[key]


def _tile_group(a):
    """[BC, 1024] -> [kp, g, kc, GT*P]; rows r = g*512 + t*128 + bp,
    cols c = kc*128 + kp; out[kp, g, kc, t*128+bp] = a[r, c]."""
    return np.ascontiguousarray(
        a.reshape(NG, GT, P, KC, P).transpose(4, 0, 3, 1, 2)
        .reshape(P, NG, KC, GT * P))


def _tile_w(w):
    """[n, 1024] (contraction on axis 1) -> [kp, kc, n]."""
    return np.ascontiguousarray(w.T.reshape(KC, P, -1).transpose(1, 0, 2))


def _ladder(a, scales):
    """fp8 ladder; returns the SCALED fp8 arrays (stored values)."""
    resid = a.astype(np.float32)
    out = []
    for s in scales:
        f8 = (resid * s).astype(F8_NP)
        out.append(f8)
        resid = resid - f8.astype(np.float32) / s
    return out


def prepare_in_maps(inputs):
    x = np.asarray(inputs["x"], np.float32)
    h = np.asarray(inputs["h_prev"], np.float32)
    memory = np.asarray(inputs["memory"], np.float32)
    W_w = np.asarray(inputs["W_w"], np.float32)
    U_w = np.asarray(inputs["U_w"], np.float32)
    Q_w = np.asarray(inputs["Q_w"], np.float32)
    M_w = np.asarray(inputs["M_w"], np.float32)
    W_b = np.asarray(inputs["W_b"], np.float32)
    U_b = np.asarray(inputs["U_b"], np.float32)
    Q_b = np.asarray(inputs["Q_b"], np.float32)
    M_b = np.asarray(inputs["M_b"], np.float32)
    ln_g = np.asarray(inputs["ln_g"], np.float32)
    ln_b = np.asarray(inputs["ln_b"], np.float32)

    mw_lv = _ladder(M_w, [1.0, 16.0, 256.0, 4096.0])
    mwp = np.zeros((P, KC, 4 * 32), dtype=F8_NP)
    for j, lv in enumerate(mw_lv):
        mwp[:, :, j * 32:j * 32 + NB] = _tile_w(
            lv.astype(np.float32)).astype(F8_NP)

    shared = {
        "W8": _tile_w(W_w).astype(F8_NP),
        "U8": _tile_w(U_w).astype(F8_NP),
        "Q8": _tile_w(Q_w).astype(F8_NP),
        "Mem8": _tile_w(memory).astype(F8_NP),
        "Mwp": mwp,
        "cb": np.ascontiguousarray(W_b + U_b + Q_b),
        "lng": np.ascontiguousarray(ln_g),
        "lnb": np.ascontiguousarray(ln_b),
        "negmb": np.ascontiguousarray(-M_b),
        "powers": (2.0 ** np.arange(NB - 1, -1, -1)).astype(np.float32),
    }
    in_maps = []
    for i in range(NCORES):
        sl = slice(i * BC, (i + 1) * BC)
        h_lv = _ladder(h[sl], [1.0, 16.0, 256.0])
        m = dict(shared)
        m["x1"] = _tile_group(x[sl].astype(F8_NP).astype(np.float32)
                              ).astype(F8_NP)
        for k, lv in zip(("h1", "h2", "h3"), h_lv):
            m[k] = _tile_group(lv.astype(np.float32)).astype(F8_NP)
        in_maps.append(m)
    return in_maps


def run(inputs, trace=False, trace_cores=None):
    zero_affine = bool(
        np.all(np.asarray(inputs["ln_g"], np.float32) == 1.0)
        and np.all(np.asarray(inputs["ln_b"], np.float32) == 0.0))
    zero_cb = bool(
        np.all(np.asarray(inputs["W_b"], np.float32) == 0.0)
        and np.all(np.asarray(inputs["U_b"], np.float32) == 0.0)
        and np.all(np.asarray(inputs["Q_b"], np.float32) == 0.0))
    nc = _get_nc(zero_affine, zero_cb)
    in_maps = prepare_in_maps(inputs)
    res = bass_utils.run_bass_kernel_spmd(
        nc, in_maps, core_ids=list(range(NCORES)), trace=trace,
        trace_cores=trace_cores)
    out = np.concatenate([r["y"] for r in res.results], axis=0)
    return out, res


def kernel(**inputs):
    out, _ = run(inputs)
    return out.astype(np.float32)


def enable_profiling():
    """Inject the missing antenv.axon_hooks shim so trace=True works, and
    neutralize the S3 artifact upload (zero-egress container)."""
    import sys
    import types
    try:
        import antenv.axon_hooks  # noqa: F401
    except ImportError:
        mod = types.ModuleType("antenv.axon_hooks")
        _hook = [None]
        mod.set_axon_ntff_profile_hook = lambda h: _hook.__setitem__(0, h)
        mod.get_axon_ntff_profile_hook = lambda: _hook[0]
        sys.modules["antenv.axon_hooks"] = mod
        from trn_agent_boot.trn_boot import _ntff_profile_via_ctypes
        mod.set_axon_ntff_profile_hook(
            _ntff_profile_via_ctypes("/opt/axon/libaxon_pjrt.so"))
    bass_utils.upload_artifacts = lambda d: "local://" + str(d)
